# revision 1
# baseline (speedup 1.0000x reference)
"""Trainium2 Bass kernel: single-head attention module (dense transformer).

Computes, for x [4, 4096, 256] (f32) and per-projection weights/biases:
    q = x @ Wq + bq;  k = x @ Wk + bk;  v = x @ Wv + bv
    out = softmax((q k^T) / sqrt(256)) @ v @ Wo + bo

Sharding over 8 NeuronCores: core c handles batch c//2, query half c%2.
The host rotates each core's batch so its queries are always rows 0..2047
(softmax is key-order invariant), keeping the device program identical
across cores. Each core computes K/V for its whole batch (redundant with
its pair core, which is cheap) and attention + output projection for its
2048 queries.

Per-core kernel layout (matmuls in float32r = full-rate ~fp32; every
matmul operand tile is declared float32r so its producer rounds on write,
which the BIR verifier requires):
  - x is loaded in natural [s,d] tiles (1 MiB DMAs — each dma_start costs
    ~650 ns on both the issuing sequencer and the shared HWDGE) and
    transposed on the PE (via identity) to x^T [d, s] so projections can
    contract over d on the partition axis.
  - Q^T [e, sq] and K^T [e, sk] are produced directly transposed
    (lhsT = W chunk, moving = x^T), which is the exact layout the scores
    matmul wants: S^T[sk_tile, sq] = (K^T chunk).T @ Q^T chunk.
  - Softmax over keys is computed WITHOUT max subtraction (scores here
    are bounded by ~±10, and softmax-no-max is the same function): P^T =
    exp(S^T/16) on the scalar engine straight out of PSUM.
  - The PV product accumulates out^T[e, sq] over the 32 key tiles in
    PSUM. The softmax denominator comes from a ones[128,128] stationary
    matmul over DVE-computed sums of four P^T tiles (the quad-sum quarters
    the extra PE stream), accumulated broadcast across all partitions.
  - out^T is scaled by 1/denom (DVE) and fed as the stationary operand of
    the final projection, which lands the output in natural [sq, f]
    layout for contiguous paired 256-row output DMAs.

Measured: rel err 2.9e-04 vs the fp32 reference on TRN2 (f32r rounding,
matches a TF32-emulation estimate). Cost-model exec 169 us/core: ~143 us
TensorE busy (87% saturated; scores 55 + PV 55 + denom 7 + projections 17
+ transposes 10), ~97 us ACT (exp), ~91 us DVE. Remaining non-PE time is
startup DMA (~3.5 us), the fixed end-of-kernel drain barrier (~4 us), the
last block's reciprocal chain (~3 us), and scattered sub-200 ns semaphore
latencies. Next levers if iterating further (needs a real neuron-profile
trace): verify f32r matmuls hit 1 cycle/row on HW back-to-back, and
whether the scores->exp->PV chain holds PE saturation under real ACT
latencies.
"""

import numpy as np

import concourse.bass as bass  # noqa: F401  (AP types come through tile/bacc)
import concourse.tile as tile
from concourse import bacc, mybir
from concourse.bass_utils import run_bass_kernel_spmd
from concourse.masks import make_identity

B, S, D = 4, 4096, 256
SQ = S // 2  # queries per core
NCORES = 8
F32 = mybir.dt.float32
F32R = mybir.dt.float32r
SCALE = 1.0 / 16.0  # 1/sqrt(D)


def _r(ap):
    """View an fp32 AP as float32r: full-rate fp32 matmul on the PE."""
    return ap.bitcast(F32R)


def _build(phases=3):
    nc = bacc.Bacc("TRN2", target_bir_lowering=False, debug=False,
                   num_devices=NCORES)

    xkv = nc.dram_tensor("xkv", [S, D], F32, kind="ExternalInput").ap()
    w_dram = {
        n: nc.dram_tensor(n, [D, D], F32, kind="ExternalInput").ap()
        for n in ("wq", "wk", "wv", "wo")
    }
    b_dram = {
        n: nc.dram_tensor(n, [D], F32, kind="ExternalInput").ap()
        for n in ("bq", "bk", "bo")
    }
    out = nc.dram_tensor("out", [SQ, D], F32, kind="ExternalOutput").ap()

    bq_col = b_dram["bq"].rearrange("(a b) -> a b", b=1)  # [256, 1]
    bk_col = b_dram["bk"].rearrange("(a b) -> a b", b=1)
    bo_row = b_dram["bo"].rearrange("(a b) -> a b", a=1)  # [1, 256]
    # Grouped views for wide DMAs: one instruction per ~1 MiB, since each
    # dma_start costs ~650 ns on the issuing sequencer AND on the shared HWDGE.
    xkv_g = xkv.rearrange("(g j p) c -> g p j c", j=8, p=128)   # [4,128,8,256]
    w_g = {n: w.rearrange("(j p) c -> p j c", j=2) for n, w in w_dram.items()}
    out_g = out.rearrange("(g j p) c -> g p j c", j=2, p=128)   # [8,128,2,256]

    with tile.TileContext(nc) as tc:
        with (
            tc.tile_pool(name="const", bufs=1) as cpool,
            tc.tile_pool(name="xin", bufs=4) as xin_pool,
            tc.tile_pool(name="pt", bufs=4) as pt_pool,
            tc.tile_pool(name="ovec", bufs=2) as ovec_pool,
            tc.tile_pool(name="fout", bufs=2) as fout_pool,
            tc.tile_pool(name="psmm", bufs=1, space="PSUM") as psmm,
            tc.tile_pool(name="psacc", bufs=1, space="PSUM") as psacc,
        ):
            # ---- constants ----
            ident = cpool.tile([128, 128], F32, tag="ident", name="ident")
            make_identity(nc, ident[:])
            ident_r = cpool.tile([128, 128], F32R, tag="identr", name="identr")
            nc.vector.tensor_copy(ident_r[:], ident[:])
            ones128 = cpool.tile([128, 128], F32R, tag="ones128", name="ones128")
            # memset can't target f32r; write the 1.0f bit pattern via uint32
            nc.vector.memset(ones128[:].bitcast(mybir.dt.uint32), 0x3F800000)
            ones1 = cpool.tile([1, 128], F32, tag="ones1", name="ones1")
            nc.vector.memset(ones1[:], 1.0)

            # ---- x DMAs first: everything depends on x, so it must win the
            # HWDGE queue ahead of the constant loads. Group 0 is split so the
            # first transposes can start after ~0.25 MiB.
            xt_tiles = []
            for g in range(4):
                xt = xin_pool.tile([128, 8 * D], F32R, tag="xin", name="xin")
                xt_j = xt.rearrange("p (j c) -> p j c", j=8)
                if g == 0:
                    nc.sync.dma_start(xt_j[:, 0:2], _r(xkv_g[g][:, 0:2]))
                    nc.sync.dma_start(xt_j[:, 2:8], _r(xkv_g[g][:, 2:8]))
                else:
                    nc.sync.dma_start(xt_j, _r(xkv_g[g]))
                xt_tiles.append(xt)

            w_sb = {}
            for n in ("wq", "wk", "wv", "wo"):
                t = cpool.tile([128, 2 * D], F32R, tag=f"w_{n}", name=f"w_{n}")
                nc.sync.dma_start(
                    t.rearrange("p (j c) -> p j c", j=2), _r(w_g[n][:]))
                w_sb[n] = t

            def wchunk(n, c):  # [128, 256] d-chunk c of W
                return w_sb[n][:, c * D:(c + 1) * D]

            bqc, bkc = [], []
            for c in range(2):
                t = cpool.tile([128, 1], F32, tag=f"bq{c}", name=f"bq{c}")
                nc.sync.dma_start(t[:], bq_col[c * 128:(c + 1) * 128, :])
                bqc.append(t)
                t = cpool.tile([128, 1], F32, tag=f"bk{c}", name=f"bk{c}")
                nc.sync.dma_start(t[:], bk_col[c * 128:(c + 1) * 128, :])
                bkc.append(t)

            # bo broadcast across partitions: ones1[1,128].T @ bo_row[1,256],
            # then duplicated side by side so one [128,512] add covers two
            # output row-tiles. (bv is folded into bo host-side: attention
            # rows sum to 1, so attn@(v+bv)@Wo + bo == attn@v@Wo + (bv@Wo+bo).)
            bob = cpool.tile([128, 2 * D], F32, tag="bob", name="bob")
            row = cpool.tile([1, D], F32, tag="bor", name="bor")
            nc.sync.dma_start(row[:], bo_row[:])
            bps = psmm.tile([128, D], F32, tag="fp", name="fp", bufs=1)
            nc.tensor.matmul(bps[:], ones1[:], row[:], start=True, stop=True)
            nc.vector.tensor_copy(bob[:, 0:D], bps[:])
            nc.vector.tensor_copy(bob[:, D:2 * D], bps[:])

            # ---- persistent activations ----
            xkvT = [cpool.tile([128, S], F32R, tag=f"xkvT{c}", name=f"xkvT{c}")
                    for c in range(2)]
            qT = [cpool.tile([128, SQ], F32R, tag=f"qT{c}", name=f"qT{c}")
                  for c in range(2)]
            kT = [cpool.tile([128, S], F32R, tag=f"kT{c}", name=f"kT{c}")
                  for c in range(2)]
            v_sb = cpool.tile([128, 32 * D], F32R, tag="v", name="v")

            # ---- phase 1: load x (1 MiB DMAs), transpose to x^T ----
            # Four 128x128 transposes land in one [128,512] PSUM bank; the
            # single wide eviction alternates between DVE and ACT so neither
            # engine becomes the phase bottleneck.
            evict_parity = 0
            for dst, ngrp in ((xkvT, 4),):
                for g in range(ngrp):
                    xt = xt_tiles[g]
                    for half in range(2):
                        for c in range(2):
                            tp = psmm.tile([128, 512], F32, tag="sc",
                                           name="sc", bufs=4)
                            for j in range(4):
                                jj = half * 4 + j
                                nc.tensor.transpose(
                                    _r(tp[:, j * 128:(j + 1) * 128]),
                                    xt[:, jj * D + c * 128:
                                       jj * D + (c + 1) * 128],
                                    ident_r[:])
                            col0 = (g * 8 + half * 4) * 128
                            dsl = dst[c][:, col0:col0 + 512]
                            if evict_parity % 2 == 0:
                                nc.vector.tensor_copy(dsl, tp[:])
                            else:
                                nc.scalar.copy(dsl, tp[:])
                            evict_parity += 1

            # ---- phase 2: projections ----
            # Q^T / K^T: lhsT = W[d_chunk, e_tile], moving = x^T[d_chunk, s]
            for (wn, xT, dstT, bcol, stot) in () if phases < 2 else (
                ("wq", xkvT, qT, bqc, SQ),
                ("wk", xkvT, kT, bkc, S),
            ):
                for et in range(2):
                    for blk in range(stot // 512):
                        pp = psmm.tile([128, 512], F32, tag="sc", name="sc",
                                       bufs=4)
                        for c in range(2):
                            nc.tensor.matmul(
                                pp[:],
                                _r(wchunk(wn, c)[:, et * 128:(et + 1) * 128]),
                                _r(xT[c][:, blk * 512:(blk + 1) * 512]),
                                start=(c == 0), stop=(c == 1),
                            )
                        dsl = dstT[et][:, blk * 512:(blk + 1) * 512]
                        if evict_parity % 2 == 0:
                            nc.vector.tensor_scalar_add(dsl, pp[:], bcol[et][:])
                        else:
                            nc.scalar.activation(
                                dsl, pp[:],
                                mybir.ActivationFunctionType.Identity,
                                bias=bcol[et][:])
                        evict_parity += 1

            # V: natural layout [sk, e]; lhsT = x^T[d_chunk, sk_tile].
            # Two sk-tiles share one [128,512] PSUM bank -> one wide eviction.
            for stp in range(16 if phases >= 2 else 0):
                vp = psmm.tile([128, 512], F32, tag="sc", name="sc", bufs=4)
                for half in range(2):
                    st = stp * 2 + half
                    for c in range(2):
                        nc.tensor.matmul(
                            vp[:, half * D:(half + 1) * D],
                            _r(xkvT[c][:, st * 128:(st + 1) * 128]),
                            _r(wchunk("wv", c)),
                            start=(c == 0), stop=(c == 1),
                        )
                dsl = v_sb[:, stp * 512:(stp + 1) * 512]
                if evict_parity % 2 == 0:
                    nc.vector.tensor_copy(dsl, vp[:])
                else:
                    nc.scalar.copy(dsl, vp[:])
                evict_parity += 1

            # ---- phase 3: attention ----
            for qb in range(SQ // 512 if phases >= 3 else 0):
                qsl = slice(qb * 512, (qb + 1) * 512)
                acc = [psacc.tile([128, 512], F32, tag=f"acc{e}",
                                  name=f"acc{e}") for e in range(2)]
                accd = psacc.tile([128, 512], F32, tag="accd", name="accd")
                ptq = []
                for st in range(32):
                    ssl = slice(st * 128, (st + 1) * 128)
                    sp = psmm.tile([128, 512], F32, tag="sc", name="sc",
                                   bufs=4)
                    nc.tensor.matmul(sp[:], _r(kT[0][:, ssl]),
                                     _r(qT[0][:, qsl]), start=True, stop=False)
                    nc.tensor.matmul(sp[:], _r(kT[1][:, ssl]),
                                     _r(qT[1][:, qsl]), start=False, stop=True)
                    pt = pt_pool.tile([128, 512], F32R, tag="pt", name="pt", bufs=6)
                    nc.scalar.activation(pt[:], sp[:],
                                         mybir.ActivationFunctionType.Exp,
                                         scale=SCALE)
                    first, last = (st == 0), (st == 31)
                    nc.tensor.matmul(acc[0][:], _r(v_sb[:, st * D:st * D + 128]),
                                     _r(pt[:]), start=first, stop=last)
                    nc.tensor.matmul(acc[1][:],
                                     _r(v_sb[:, st * D + 128:(st + 1) * D]),
                                     _r(pt[:]), start=first, stop=last)
                    # Denominator: sum pt quads on DVE (off the PE's
                    # critical path), quartering the ones-matmul streams.
                    ptq.append(pt)
                    if st % 4 == 3:
                        pa = pt_pool.tile([128, 512], F32R, tag="ptsum",
                                          name="ptsum")
                        nc.vector.tensor_add(pa[:], ptq[0][:], ptq[1][:])
                        pb = pt_pool.tile([128, 512], F32R, tag="ptsum",
                                          name="ptsum")
                        nc.vector.tensor_add(pb[:], ptq[2][:], ptq[3][:])
                        pc = pt_pool.tile([128, 512], F32R, tag="ptsum",
                                          name="ptsum")
                        nc.vector.tensor_add(pc[:], pa[:], pb[:])
                        nc.tensor.matmul(accd[:], _r(ones128[:]), _r(pc[:]),
                                         start=(st == 3), stop=(st == 31))
                        ptq = []

                rec = ovec_pool.tile([128, 512], F32, tag="rec", name="rec")
                o = [ovec_pool.tile([128, 512], F32R, tag=f"o{e}",
                                    name=f"o{e}") for e in range(2)]
                # halves: lets the first final matmuls start ~0.8us earlier
                for hsl in (slice(0, 256), slice(256, 512)):
                    nc.vector.reciprocal(rec[:, hsl], accd[:, hsl])
                    for e in range(2):
                        nc.vector.tensor_mul(o[e][:, hsl], acc[e][:, hsl],
                                             rec[:, hsl])

                # Final projection: two row-tiles per [128,512] staging tile,
                # one paired 256-row output DMA.
                for pair in range(2):
                    fo = fout_pool.tile([128, 2 * D], F32, tag="fout",
                                        name="fout")
                    for half in range(2):
                        t4 = pair * 2 + half
                        tsl = slice(t4 * 128, (t4 + 1) * 128)
                        fp = psmm.tile([128, D], F32, tag="fp", name="fp",
                                       bufs=1)
                        for e in range(2):
                            nc.tensor.matmul(fp[:], _r(o[e][:, tsl]),
                                             _r(wchunk("wo", e)),
                                             start=(e == 0), stop=(e == 1))
                        nc.vector.tensor_add(fo[:, half * D:(half + 1) * D],
                                             fp[:], bob[:, 0:D])
                    nc.sync.dma_start(out_g[qb * 2 + pair],
                                      fo.rearrange("p (j c) -> p j c", j=2))

    nc.compile()
    return nc



_NC = None


def _get_nc():
    global _NC
    if _NC is None:
        _NC = _build()
    return _NC


class _Runner:
    """Cached jitted SPMD executor (run_bass_kernel_spmd rebuilds its jax
    closure every call, forcing a retrace; this traces once)."""

    def __init__(self, nc):
        import jax
        from jax.sharding import Mesh, PartitionSpec
        from jax.experimental.shard_map import shard_map
        from concourse import bass2jax, mybir as mb

        bass2jax.install_neuronx_cc_hook()
        self.jax = jax
        if not any("axon" in str(getattr(d, "platform", "")).lower()
                   or str(d).startswith("NC_")
                   for d in jax.devices()):
            # jax was initialized on another platform (e.g. cpu for the
            # reference); reset so the axon NeuronCores are visible.
            import jax._src.xla_bridge as xb
            jax.config.update("jax_platforms", None)
            xb._clear_backends()
            if hasattr(xb.get_backend, "cache_clear"):
                xb.get_backend.cache_clear()
            if not any("axon" in str(getattr(d, "platform", "")).lower()
                       or str(d).startswith("NC_")
                       for d in jax.devices()):
                jax.config.update("jax_platforms", "axon")
                xb._clear_backends()
                if hasattr(xb.get_backend, "cache_clear"):
                    xb.get_backend.cache_clear()
        partition_name = (nc.partition_id_tensor.name
                          if nc.partition_id_tensor else None)
        in_names, out_names, out_avals = [], [], []
        for alloc in nc.m.functions[0].allocations:
            if not isinstance(alloc, mb.MemoryLocationSet):
                continue
            name = alloc.memorylocations[0].name
            if alloc.kind == "ExternalInput":
                if name != partition_name:
                    in_names.append(name)
            elif alloc.kind == "ExternalOutput":
                out_names.append(name)
                out_avals.append(jax.core.ShapedArray(
                    tuple(alloc.tensor_shape), mb.dt.np(alloc.dtype)))
        self.in_names, self.out_names, self.out_avals = \
            in_names, out_names, out_avals
        n_params, n_outs = len(in_names), len(out_names)
        bind_in_names = in_names + out_names + (
            [partition_name] if partition_name else [])

        def _body(*args):
            operands = list(args)
            if partition_name is not None:
                operands.append(bass2jax.partition_id_tensor())
            outs = bass2jax._bass_exec_p.bind(
                *operands,
                out_avals=tuple(out_avals),
                in_names=tuple(bind_in_names),
                out_names=tuple(out_names),
                lowering_input_output_aliases=(),
                sim_require_finite=True,
                sim_require_nnan=True,
                nc=nc,
            )
            return tuple(outs)

        devices = jax.devices()[:NCORES]
        mesh = Mesh(np.asarray(devices), ("core",))
        spec = (PartitionSpec("core"),) * (n_params + n_outs)
        self.fn = jax.jit(
            shard_map(_body, mesh=mesh, in_specs=spec,
                      out_specs=(PartitionSpec("core"),) * n_outs,
                      check_rep=False),
            donate_argnums=tuple(range(n_params, n_params + n_outs)),
            keep_unused=True,
        )

    def run(self, in_maps):
        concat_in = [
            np.concatenate([np.asarray(m[n]) for m in in_maps], axis=0)
            for n in self.in_names
        ]
        concat_zeros = [
            np.zeros((NCORES * a.shape[0], *a.shape[1:]), a.dtype)
            for a in self.out_avals
        ]
        outs = self.fn(*concat_in, *concat_zeros)
        return [
            {n: np.asarray(outs[i]).reshape(NCORES, *self.out_avals[i].shape)[c]
             for i, n in enumerate(self.out_names)}
            for c in range(NCORES)
        ]


_RUNNER = None


def _get_runner():
    global _RUNNER
    if _RUNNER is None:
        _RUNNER = _Runner(_get_nc())
    return _RUNNER


def kernel(**inputs):
    x = np.ascontiguousarray(np.asarray(inputs["x"], dtype=np.float32))
    Wq = np.ascontiguousarray(np.asarray(inputs["Wq"], dtype=np.float32))
    Wk = np.ascontiguousarray(np.asarray(inputs["Wk"], dtype=np.float32))
    Wv = np.ascontiguousarray(np.asarray(inputs["Wv"], dtype=np.float32))
    Wo = np.ascontiguousarray(np.asarray(inputs["Wo"], dtype=np.float32))
    bq = np.ascontiguousarray(np.asarray(inputs["bq"], dtype=np.float32))
    bk = np.ascontiguousarray(np.asarray(inputs["bk"], dtype=np.float32))
    bv = np.ascontiguousarray(np.asarray(inputs["bv"], dtype=np.float32))
    bo = np.ascontiguousarray(np.asarray(inputs["bo"], dtype=np.float32))

    try:
        runner = _get_runner()
    except Exception:
        runner = None
    # bv folds into bo: attention rows sum to 1, so attn@(v+bv) = attn@v + bv.
    bo_eff = (bv @ Wo + bo).astype(np.float32)
    in_maps = []
    for c in range(NCORES):
        b, h = divmod(c, 2)
        # Rotate the batch so this core's queries are rows 0..SQ-1; keys and
        # values see all rows either way (softmax is key-order invariant).
        xb = x[b] if h == 0 else np.ascontiguousarray(
            np.concatenate([x[b, SQ:], x[b, :SQ]]))
        in_maps.append({
            "xkv": xb,
            "wq": Wq, "wk": Wk, "wv": Wv, "wo": Wo,
            "bq": bq, "bk": bk, "bo": bo_eff,
        })
    results = None
    if runner is not None:
        try:
            results = runner.run(in_maps)
        except Exception:
            results = None
    if results is None:
        results = run_bass_kernel_spmd(
            _get_nc(), in_maps, core_ids=list(range(NCORES))).results
    outp = np.empty((B, S, D), dtype=np.float32)
    for c in range(NCORES):
        b, h = divmod(c, 2)
        outp[b, h * SQ:(h + 1) * SQ] = results[c]["out"]
    return outp



# revision 10
# speedup vs baseline: 1.3201x; 1.3201x over previous
"""Trainium2 Bass kernel: single-head attention module (dense transformer).

Computes, for x [4, 4096, 256] (f32) and per-projection weights/biases:
    q = x @ Wq + bq;  k = x @ Wk + bk;  v = x @ Wv + bv
    out = softmax((q k^T) / sqrt(256)) @ v @ Wo + bo

Sharding over 8 NeuronCores: core c handles batch c//2, query half c%2.
The host rotates each core's batch so its queries are always rows 0..2047
(softmax is key-order invariant), keeping the device program identical
across cores.

Math rewrite (host-side, weights only):
  scores = (x Wq + bq)(x Wk + bk)^T / 16
         = x (Wq Wk^T / 16) x^T + per-query const (softmax-invariant)
           + per-key term x_k . (Wk bq) (folded into the exp bias)
  so the device computes A = x M (M = 16 Wq Wk^T, sigma~1), scoresT = A x^T
  with x^T stationary, and exp(scoresT/256 + cvec). bv/bo fold into the
  output bias. The softmax division commutes past Wo:
  (num/den) @ Wo = diag(1/den) (num @ Wo), so the final projection runs on
  the unnormalized numerator and the per-query 1/den is applied on the
  output eviction (scalar_tensor_tensor: fp * rec + bias), keeping the
  reciprocal off the tail's critical path.

Precision strategy (rel-err budget 2e-2; this lands ~3e-3):
  - fp8(e4m3) DoubleRow matmuls run 2 rows/cycle with K=256 per pass.
    Every fp8 operand is split hi/lo (lo = fp8 of the residual; operands
    pre-scaled to sigma~1 so residuals clear the subnormal floor) and
    products use the 3-term expansion ah*bh + al*bh + ah*bl: bf16 accuracy
    at 0.75x the bf16 row count. x is split host-side; A on-chip during
    PSUM eviction (ACT copy -> ah, DVE subtract -> al).
  - exp output, PV, denominator sums, final projection: bf16.
  - v is computed as 16 v (Wv pre-scaled); the denominator matmul uses a
    16.0 stationary, so out = (p . 16v) / (16 sum p) exactly.
  - The denominator needs a per-QUERY-partition layout for the fused
    output eviction; a [128,512] broadcast tile is transposed on the PE
    (bf16, 4x128 rows) and reciprocal'd as a [128,4] strided read.

Schedule (single Tile context, PE kept back-to-back):
  - One packed byte-DMA carries all small weights; x arrives in 6 pieces
    sized so A-block 0 starts ~3 us in.
  - V-projection pairs and later A-projection blocks are interleaved into
    the attention loops (V inside qb0 two tiles ahead of the PV that
    consumes it; A block b+1 inside qb b), so their PSUM evictions hide
    under scores/PV instead of serializing before the loop.
  - Denominator: DVE oct-tree sums (bf16 2x) + one 16.0-matmul per 8
    key-tiles.

Per-core PE: A 6.1k + V 12.3k + scores 98.3k + PV 131.1k + denom 8.2k
+ den-transpose 2k + final 8.2k ~= 266k cycles ~= 111 us at 2.4 GHz
(f32r baseline: 343k = 143 us). ACT ~27 us/qb worst, DVE ~19 us/qb
worst, both under the PE's ~28 us/qb.
"""

import numpy as np
import ml_dtypes

import concourse.bass as bass  # noqa: F401  (AP types come through tile/bacc)
import concourse.tile as tile
from concourse import bacc, mybir
from concourse.bass_utils import run_bass_kernel_spmd

B, S, D = 4, 4096, 256
SQ = S // 2  # queries per core
NCORES = 8
F32 = mybir.dt.float32
BF16 = mybir.dt.bfloat16
F8 = mybir.dt.float8e4
U8 = mybir.dt.uint8
EXP_SCALE = 1.0 / 256.0  # 1/sqrt(D) folded with the 16x M scaling
E4M3 = ml_dtypes.float8_e4m3
DR = mybir.MatmulPerfMode.DoubleRow
ALU = mybir.AluOpType

# wp2b packed byte offsets (per partition)
WP2_WO, WP2_CVEC, WP2_BOB, WP2_IDENT, WP2_END = (0, 1024, 1152, 2176, 2432)


def _build(phases=3):
    nc = bacc.Bacc("TRN2", target_bir_lowering=False, debug=False,
                   num_devices=NCORES)

    # x^T hi/lo splits, chunk-stacked and packed: per partition row d_lo,
    # [xh chunk0 sk | xh chunk1 sk | xl chunk0 sk | xl chunk1 sk] fp8
    xpk_d = nc.dram_tensor("xpk", [128, 4 * S], F8, kind="ExternalInput").ap()
    # wpa: mh|ml|wvh|wvl packed fp8 [d_lo 128, (c 2, e 256)] each
    wpa_d = nc.dram_tensor("wpa", [128, 2048], U8, kind="ExternalInput").ap()
    # wp2b: wo|cvec|bob|ident (see WP2_* offsets)
    wp2b_d = nc.dram_tensor("wp2b", [128, WP2_END], U8,
                            kind="ExternalInput").ap()
    out = nc.dram_tensor("out", [SQ, D], F32, kind="ExternalOutput").ap()

    out_g = out.rearrange("(g j p) c -> g p j c", j=2, p=128)  # [8,128,2,256]
    xpk_g = xpk_d.rearrange("p (h c s) -> p h c s", h=2, c=2)

    with tile.TileContext(nc) as tc:
        with (
            tc.tile_pool(name="const", bufs=1) as cpool,
            tc.tile_pool(name="pt", bufs=10) as pt_pool,
            tc.tile_pool(name="pts", bufs=8) as pts_pool,
            tc.tile_pool(name="ovec", bufs=2) as ovec_pool,
            tc.tile_pool(name="fout", bufs=2) as fout_pool,
            tc.tile_pool(name="psmm", bufs=1, space="PSUM") as psmm,
            tc.tile_pool(name="psacc", bufs=1, space="PSUM") as psacc,
        ):
            # ---- input DMAs, ordered so A block 0 can start ~3 us in ----
            xpk = cpool.tile([128, 4 * S], F8, tag="xpk", name="xpk")
            xpk_4 = xpk[:].rearrange("p (h c s) -> p h c s", h=2, c=2)
            xht_3 = xpk[:, 0:2 * S].rearrange("p (c s) -> p c s", c=2)
            xlt_3 = xpk[:, 2 * S:4 * S].rearrange("p (c s) -> p c s", c=2)
            wpa = cpool.tile([128, 2048], U8, tag="wpa", name="wpa")
            wp2b = cpool.tile([128, WP2_END], U8, tag="wp2b", name="wp2b")

            nc.sync.dma_start(wpa[:], wpa_d)
            nc.sync.dma_start(xpk_4[:, :, :, 0:512], xpk_g[:, :, :, 0:512])
            nc.sync.dma_start(xpk_4[:, :, :, 512:SQ],
                              xpk_g[:, :, :, 512:SQ])
            nc.sync.dma_start(wp2b[:], wp2b_d)
            nc.sync.dma_start(xpk_4[:, :, :, SQ:S], xpk_g[:, :, :, SQ:S])

            mh = wpa[:, 0:512].bitcast(F8).rearrange("p (c e) -> p c e", c=2)
            ml = wpa[:, 512:1024].bitcast(F8).rearrange(
                "p (c e) -> p c e", c=2)
            wvh = wpa[:, 1024:1536].bitcast(F8).rearrange(
                "p (c e) -> p c e", c=2)
            wvl = wpa[:, 1536:2048].bitcast(F8).rearrange(
                "p (c e) -> p c e", c=2)
            wo_3 = wp2b[:, WP2_WO:WP2_CVEC].bitcast(BF16).rearrange(
                "p (c e) -> p c e", c=2)
            cvec = wp2b[:, WP2_CVEC:WP2_BOB].bitcast(F32)     # [128, 32]
            bob = wp2b[:, WP2_BOB:WP2_IDENT].bitcast(F32)     # [128, 256]
            ident = wp2b[:, WP2_IDENT:WP2_END].bitcast(BF16)  # [128, 128]

            ones16 = cpool.tile([128, 128], BF16, tag="ones16", name="ones16")
            # 16.0 in bf16 is 0x4180
            nc.vector.memset(ones16[:].bitcast(mybir.dt.uint16), 0x4180)

            # ---- persistent activations ----
            ahT = cpool.tile([128, 2 * SQ], F8, tag="ahT", name="ahT")
            alT = cpool.tile([128, 2 * SQ], F8, tag="alT", name="alT")
            ahT_3 = ahT[:].rearrange("p (c q) -> p c q", c=2)
            alT_3 = alT[:].rearrange("p (c q) -> p c q", c=2)
            v_sb = cpool.tile([128, 32 * D], BF16, tag="v", name="v")

            def emit_ablk_et(blk, et):
                """One e-tile of A^T q-block: 3-term fp8 DoubleRow matmuls
                plus hi/lo split eviction (ACT copy, DVE subtract)."""
                qsl = slice(blk * 512, (blk + 1) * 512)
                esl = slice(et * 128, (et + 1) * 128)
                pp = psmm.tile([128, 512], F32, tag="sc", name="sc", bufs=5)
                nc.tensor.matmul(pp[:], mh[:, :, esl], xht_3[:, :, qsl],
                                 start=True, stop=False, perf_mode=DR)
                nc.tensor.matmul(pp[:], ml[:, :, esl], xht_3[:, :, qsl],
                                 start=False, stop=False, perf_mode=DR)
                nc.tensor.matmul(pp[:], mh[:, :, esl], xlt_3[:, :, qsl],
                                 start=False, stop=True, perf_mode=DR)
                nc.scalar.copy(ahT_3[:, et, qsl], pp[:])
                nc.vector.tensor_sub(alT_3[:, et, qsl], pp[:],
                                     ahT_3[:, et, qsl])

            vparity = [0]

            def emit_vpair(stp):
                """v16 for sk-tiles 2stp,2stp+1: 3-term fp8 DR, one bank."""
                vp = psmm.tile([128, 512], F32, tag="sc", name="sc", bufs=5)
                for half in range(2):
                    st = stp * 2 + half
                    ssl = slice(st * 128, (st + 1) * 128)
                    osl = slice(half * D, (half + 1) * D)
                    nc.tensor.matmul(vp[:, osl], xht_3[:, :, ssl], wvh[:],
                                     start=True, stop=False, perf_mode=DR)
                    nc.tensor.matmul(vp[:, osl], xlt_3[:, :, ssl], wvh[:],
                                     start=False, stop=False, perf_mode=DR)
                    nc.tensor.matmul(vp[:, osl], xht_3[:, :, ssl], wvl[:],
                                     start=False, stop=True, perf_mode=DR)
                dsl = v_sb[:, stp * 512:(stp + 1) * 512]
                if vparity[0] % 2 == 0:
                    nc.vector.tensor_copy(dsl, vp[:])
                else:
                    nc.scalar.copy(dsl, vp[:])
                vparity[0] += 1

            if phases >= 1:
                emit_ablk_et(0, 0)
                emit_ablk_et(0, 1)
                emit_vpair(0)
                emit_vpair(1)

            # ---- attention ----
            # The per-block tail (den^T/rec, numerator evictions, final
            # projection) is software-pipelined into the next block's first
            # iterations so its eviction latencies hide under scores/PV.
            def tail_part1(p, terminal=False):
                """Evictions of accd and the numerators; frees all psacc
                banks for the next block. den first: the PE transposes are
                its only consumer and come earliest."""
                den = ovec_pool.tile([128, 512], BF16, tag="den", name="den")
                nc.scalar.copy(den[:], p["accd"][:])
                p["den"] = den
                o0 = ovec_pool.tile([128, 512], BF16, tag="o0", name="o0")
                nc.scalar.copy(o0[:], p["acc"][0][:])
                o1 = ovec_pool.tile([128, 512], BF16, tag="o1", name="o1")
                nc.vector.tensor_copy(o1[:], p["acc"][1][:])
                p["o"] = (o0, o1)

            def tail_part2(p):
                """den^T on the PE (bf16), 1/den as a [128,4] strided read."""
                scd = psmm.tile([128, 512], F32, tag="sc", name="sc", bufs=5)
                scd_bf = scd[:, 0:256].bitcast(BF16)
                den = p["den"]
                for t4 in range(4):
                    nc.tensor.transpose(scd_bf[:, t4 * 128:(t4 + 1) * 128],
                                        den[:, t4 * 128:(t4 + 1) * 128],
                                        ident)
                rec = ovec_pool.tile([128, 4], F32, tag="rec", name="rec")
                nc.vector.reciprocal(
                    rec[:],
                    scd_bf.rearrange("p (b c) -> p b c", c=128)[:, :, 0])
                p["rec"] = rec

            def tail_part3(p):
                """fp = o Wo into the freed accd bank + one sc slot; evict
                with the fused 1/den scale and output bias; DMA out."""
                fp4a = psacc.tile([128, 512], F32, tag="accd", name="accd")
                scx = psmm.tile([128, 512], F32, tag="sc", name="sc", bufs=5)
                fp_slices = [fp4a[:, 0:256], fp4a[:, 256:512],
                             scx[:, 0:256], scx[:, 256:512]]
                o, rec = p["o"], p["rec"]
                for pair in range(2):
                    fo = fout_pool.tile([128, 2 * D], F32, tag="fout",
                                        name="fout")
                    for half in range(2):
                        t4 = pair * 2 + half
                        tsl = slice(t4 * 128, (t4 + 1) * 128)
                        fp = fp_slices[t4]
                        for e in range(2):
                            nc.tensor.matmul(fp, o[e][:, tsl], wo_3[:, e, :],
                                             start=(e == 0), stop=(e == 1))
                        nc.vector.scalar_tensor_tensor(
                            fo[:, half * D:(half + 1) * D], fp,
                            rec[:, t4:t4 + 1], bob,
                            op0=ALU.mult, op1=ALU.add)
                    nc.sync.dma_start(out_g[p["qb"] * 2 + pair],
                                      fo.rearrange("p (j c) -> p j c", j=2))

            # Producer/consumer skew: scores+exp run LEAD tiles ahead of the
            # PV+denominator consumer, across qb boundaries, so ACT's exp
            # pipeline never restarts from empty when a new block begins.
            LEAD = 2
            state = {"acc": None, "accd": None, "pending": None,
                     "ptq": [], "l1q": [], "l2q": []}

            def consume(cqb, cst, pt):
                if cst == 0:
                    if state["pending"] is not None:
                        tail_part1(state["pending"])
                    state["acc"] = [
                        psacc.tile([128, 512], F32, tag=f"acc{e}",
                                   name=f"acc{e}") for e in range(2)]
                    state["accd"] = None
                acc = state["acc"]
                first, last = (cst == 0), (cst == 31)
                nc.tensor.matmul(acc[0][:], v_sb[:, cst * D:cst * D + 128],
                                 pt[:], start=first, stop=last)
                nc.tensor.matmul(acc[1][:],
                                 v_sb[:, cst * D + 128:(cst + 1) * D],
                                 pt[:], start=first, stop=last)
                p = state["pending"]
                if p is not None:
                    if cst == 1:
                        tail_part2(p)
                    elif cst == 2:
                        tail_part3(p)
                        state["pending"] = None
                # Denominator: incremental oct-tree sums on DVE (bf16 2x
                # mode) as pairs complete, then one 16.0-stationary matmul
                # per 8 tiles.
                ptq, l1q, l2q = state["ptq"], state["l1q"], state["l2q"]
                ptq.append(pt)
                if cst % 2 == 1:
                    ps = pts_pool.tile([128, 512], BF16, tag="pts",
                                       name="pts")
                    nc.vector.tensor_add(ps[:], ptq[-2][:], ptq[-1][:])
                    l1q.append(ps)
                if cst % 4 == 3:
                    ps = pts_pool.tile([128, 512], BF16, tag="pts",
                                       name="pts")
                    nc.vector.tensor_add(ps[:], l1q[-2][:], l1q[-1][:])
                    l2q.append(ps)
                if cst % 8 == 7:
                    ps = pts_pool.tile([128, 512], BF16, tag="pts",
                                       name="pts")
                    nc.vector.tensor_add(ps[:], l2q[-2][:], l2q[-1][:])
                    if state["accd"] is None:
                        state["accd"] = psacc.tile([128, 512], F32,
                                                   tag="accd", name="accd")
                    nc.tensor.matmul(state["accd"][:], ones16[:], ps[:],
                                     start=(cst == 7), stop=(cst == 31))
                    state["ptq"], state["l1q"], state["l2q"] = [], [], []
                if cst == 31:
                    state["pending"] = {"acc": acc, "accd": state["accd"],
                                        "qb": cqb}

            nqb = SQ // 512 if phases >= 2 else 0
            fifo = []
            for qb in range(nqb):
                qsl = slice(qb * 512, (qb + 1) * 512)
                for st in range(32):
                    # interleaved producer work for later consumers
                    if qb == 0 and st % 2 == 0 and st // 2 + 2 <= 15:
                        emit_vpair(st // 2 + 2)
                    if qb < 3 and st in (11, 21):
                        emit_ablk_et(qb + 1, 0 if st == 11 else 1)

                    ssl = slice(st * 128, (st + 1) * 128)
                    sp = psmm.tile([128, 512], F32, tag="sc", name="sc",
                                   bufs=5)
                    nc.tensor.matmul(sp[:], xht_3[:, :, ssl],
                                     ahT_3[:, :, qsl], start=True, stop=False,
                                     perf_mode=DR)
                    nc.tensor.matmul(sp[:], xht_3[:, :, ssl],
                                     alT_3[:, :, qsl], start=False,
                                     stop=False, perf_mode=DR)
                    nc.tensor.matmul(sp[:], xlt_3[:, :, ssl],
                                     ahT_3[:, :, qsl], start=False, stop=True,
                                     perf_mode=DR)
                    pt = pt_pool.tile([128, 512], BF16, tag="pt", name="pt",
                                      bufs=10)
                    nc.scalar.activation(pt[:], sp[:],
                                         mybir.ActivationFunctionType.Exp,
                                         scale=EXP_SCALE,
                                         bias=cvec[:, st:st + 1])
                    fifo.append((qb, st, pt))
                    if len(fifo) > LEAD:
                        consume(*fifo.pop(0))
            for item in fifo:
                consume(*item)

            if state["pending"] is not None and phases >= 2:
                tail_part1(state["pending"], terminal=True)
                tail_part2(state["pending"])
                tail_part3(state["pending"])

    nc.compile()
    return nc


_NC = None


def _get_nc():
    global _NC
    if _NC is None:
        _NC = _build()
    return _NC


class _Runner:
    """Cached jitted SPMD executor (run_bass_kernel_spmd rebuilds its jax
    closure every call, forcing a retrace; this traces once)."""

    def __init__(self, nc):
        import jax
        from jax.sharding import Mesh, PartitionSpec
        from jax.experimental.shard_map import shard_map
        from concourse import bass2jax, mybir as mb

        bass2jax.install_neuronx_cc_hook()
        self.jax = jax
        if not any("axon" in str(getattr(d, "platform", "")).lower()
                   or str(d).startswith("NC_")
                   for d in jax.devices()):
            # jax was initialized on another platform (e.g. cpu for the
            # reference); reset so the axon NeuronCores are visible.
            import jax._src.xla_bridge as xb
            jax.config.update("jax_platforms", None)
            xb._clear_backends()
            if hasattr(xb.get_backend, "cache_clear"):
                xb.get_backend.cache_clear()
            if not any("axon" in str(getattr(d, "platform", "")).lower()
                       or str(d).startswith("NC_")
                       for d in jax.devices()):
                jax.config.update("jax_platforms", "axon")
                xb._clear_backends()
                if hasattr(xb.get_backend, "cache_clear"):
                    xb.get_backend.cache_clear()
        partition_name = (nc.partition_id_tensor.name
                          if nc.partition_id_tensor else None)
        in_names, out_names, out_avals = [], [], []
        for alloc in nc.m.functions[0].allocations:
            if not isinstance(alloc, mb.MemoryLocationSet):
                continue
            name = alloc.memorylocations[0].name
            if alloc.kind == "ExternalInput":
                if name != partition_name:
                    in_names.append(name)
            elif alloc.kind == "ExternalOutput":
                out_names.append(name)
                out_avals.append(jax.core.ShapedArray(
                    tuple(alloc.tensor_shape), mb.dt.np(alloc.dtype)))
        self.in_names, self.out_names, self.out_avals = \
            in_names, out_names, out_avals
        n_params, n_outs = len(in_names), len(out_names)
        bind_in_names = in_names + out_names + (
            [partition_name] if partition_name else [])

        def _body(*args):
            operands = list(args)
            if partition_name is not None:
                operands.append(bass2jax.partition_id_tensor())
            outs = bass2jax._bass_exec_p.bind(
                *operands,
                out_avals=tuple(out_avals),
                in_names=tuple(bind_in_names),
                out_names=tuple(out_names),
                lowering_input_output_aliases=(),
                sim_require_finite=True,
                sim_require_nnan=True,
                nc=nc,
            )
            return tuple(outs)

        devices = jax.devices()[:NCORES]
        mesh = Mesh(np.asarray(devices), ("core",))
        spec = (PartitionSpec("core"),) * (n_params + n_outs)
        self.fn = jax.jit(
            shard_map(_body, mesh=mesh, in_specs=spec,
                      out_specs=(PartitionSpec("core"),) * n_outs,
                      check_rep=False),
            donate_argnums=tuple(range(n_params, n_params + n_outs)),
            keep_unused=True,
        )

    def run(self, in_maps):
        concat_in = [
            np.concatenate([np.asarray(m[n]) for m in in_maps], axis=0)
            for n in self.in_names
        ]
        concat_zeros = [
            np.zeros((NCORES * a.shape[0], *a.shape[1:]), a.dtype)
            for a in self.out_avals
        ]
        outs = self.fn(*concat_in, *concat_zeros)
        return [
            {n: np.asarray(outs[i]).reshape(NCORES, *self.out_avals[i].shape)[c]
             for i, n in enumerate(self.out_names)}
            for c in range(NCORES)
        ]


_RUNNER = None


def _get_runner():
    global _RUNNER
    if _RUNNER is None:
        _RUNNER = _Runner(_get_nc())
    return _RUNNER


def _split8(a):
    """fp8 e4m3 hi/lo split: a ~= hi + lo elementwise."""
    hi = np.asarray(a, dtype=E4M3)
    lo = np.asarray(a.astype(np.float32) - hi.astype(np.float32), dtype=E4M3)
    return hi, lo


def _dstack(a):
    """[256, N] (d-major) -> [128, 2*N] chunk-stacked: row d_lo holds
    (chunk 0 cols, chunk 1 cols)."""
    n = a.shape[1]
    return np.ascontiguousarray(
        a.reshape(2, 128, n).transpose(1, 0, 2).reshape(128, 2 * n))


def make_in_maps(inputs):
    x = np.asarray(inputs["x"], dtype=np.float32)
    Wq = np.asarray(inputs["Wq"], dtype=np.float32)
    Wk = np.asarray(inputs["Wk"], dtype=np.float32)
    Wv = np.asarray(inputs["Wv"], dtype=np.float32)
    Wo = np.asarray(inputs["Wo"], dtype=np.float32)
    bq = np.asarray(inputs["bq"], dtype=np.float32)
    bv = np.asarray(inputs["bv"], dtype=np.float32)
    bo = np.asarray(inputs["bo"], dtype=np.float32)
    # bk drops out of softmax (per-query constant). bq only survives through
    # the per-key term x_k . (Wk bq), applied as an exp bias. bv folds into
    # the output bias (attention rows sum to 1).
    u8 = np.uint8
    M16 = (16.0 * (Wq @ Wk.T)).astype(np.float32)
    mh, ml = _split8(M16)
    wvh, wvl = _split8((16.0 * Wv).astype(np.float32))
    wpa = np.ascontiguousarray(np.concatenate(
        [_dstack(mh).view(u8), _dstack(ml).view(u8),
         _dstack(wvh).view(u8), _dstack(wvl).view(u8)], axis=1))
    wo_b = _dstack(np.asarray(Wo, dtype=ml_dtypes.bfloat16))
    bob = np.tile((bv @ Wo + bo).astype(np.float32)[None, :], (128, 1))
    ident = np.eye(128, dtype=ml_dtypes.bfloat16)
    wkbq = (Wk @ bq).astype(np.float32)
    in_maps = []
    for c in range(NCORES):
        b, h = divmod(c, 2)
        # Rotate the batch so this core's queries are rows 0..SQ-1; keys and
        # values see all rows either way (softmax is key-order invariant).
        xb = x[b] if h == 0 else np.ascontiguousarray(
            np.concatenate([x[b, SQ:], x[b, :SQ]]))
        xh, xl = _split8(xb)
        cvec = np.ascontiguousarray(
            (xb @ wkbq).astype(np.float32).reshape(32, 128).T)
        wp2b = np.ascontiguousarray(np.concatenate(
            [wo_b.view(u8), cvec.view(u8), bob.view(u8), ident.view(u8)],
            axis=1))
        assert wp2b.shape == (128, WP2_END)
        in_maps.append({
            "xpk": np.concatenate(
                [_dstack(np.ascontiguousarray(xh.T)),
                 _dstack(np.ascontiguousarray(xl.T))], axis=1),
            "wpa": wpa,
            "wp2b": wp2b,
        })
    return in_maps


def kernel(**inputs):
    try:
        runner = _get_runner()
    except Exception:
        runner = None
    in_maps = make_in_maps(inputs)
    results = None
    if runner is not None:
        try:
            results = runner.run(in_maps)
        except Exception:
            results = None
    if results is None:
        results = run_bass_kernel_spmd(
            _get_nc(), in_maps, core_ids=list(range(NCORES))).results
    outp = np.empty((B, S, D), dtype=np.float32)
    for c in range(NCORES):
        b, h = divmod(c, 2)
        outp[b, h * SQ:(h + 1) * SQ] = results[c]["out"]
    return outp


# revision 24
# speedup vs baseline: 1.3498x; 1.0225x over previous
"""Trainium2 Bass kernel: single-head attention module (dense transformer).

Computes, for x [4, 4096, 256] (f32) and per-projection weights/biases:
    q = x @ Wq + bq;  k = x @ Wk + bk;  v = x @ Wv + bv
    out = softmax((q k^T) / sqrt(256)) @ v @ Wo + bo

Sharding over 8 NeuronCores: core c handles batch c//2, query half c%2.
The host rotates each core's batch so its queries are always rows 0..2047
(softmax is key-order invariant), keeping the device program identical
across cores.

Math rewrite (host-side, weights only):
  scores = (x Wq + bq)(x Wk + bk)^T / 16
         = x (Wq Wk^T / 16) x^T + per-query const (softmax-invariant)
           + per-key term x_k . (Wk bq) (folded into the exp bias)
  so the device computes A = x M (M = 16 Wq Wk^T, sigma~1), scoresT = A x^T
  with x^T stationary, and exp(scoresT/256 + cvec). bv/bo fold into the
  output bias. The softmax division commutes past Wo:
  (num/den) @ Wo = diag(1/den) (num @ Wo), so the final projection runs on
  the unnormalized numerator and the per-query 1/den is applied on the
  output eviction (scalar_tensor_tensor: fp * rec + bias), keeping the
  reciprocal off the tail's critical path.

Precision strategy (rel-err budget 2e-2; this lands ~3e-3):
  - fp8(e4m3) DoubleRow matmuls run 2 rows/cycle with K=256 per pass.
    Every fp8 operand is split hi/lo (lo = fp8 of the residual; operands
    pre-scaled to sigma~1 so residuals clear the subnormal floor) and
    products use the 3-term expansion ah*bh + al*bh + ah*bl: bf16 accuracy
    at 0.75x the bf16 row count. x is split host-side; A on-chip during
    PSUM eviction (ACT copy -> ah, DVE subtract -> al).
  - exp output, PV, denominator sums, final projection: bf16.
  - v is computed as 16 v (Wv pre-scaled); the denominator matmul uses a
    16.0 stationary, so out = (p . 16v) / (16 sum p) exactly.
  - The denominator needs a per-QUERY-partition layout for the fused
    output eviction; a [128,512] broadcast tile is transposed on the PE
    (bf16, 4x128 rows) and reciprocal'd as a [128,4] strided read.

Schedule (single Tile context, PE kept back-to-back):
  - One packed byte-DMA carries all small weights; x arrives in 6 pieces
    sized so A-block 0 starts ~3 us in.
  - V-projection pairs and later A-projection blocks are interleaved into
    the attention loops (V inside qb0 two tiles ahead of the PV that
    consumes it; A block b+1 inside qb b), so their PSUM evictions hide
    under scores/PV instead of serializing before the loop.
  - Denominator: DVE oct-tree sums (bf16 2x) + one 16.0-matmul per 8
    key-tiles.

Per-core PE: A 6.1k + V 12.3k + scores 98.3k + PV 131.1k + denom 8.2k
+ den-transpose 2k + final 8.2k ~= 266k cycles ~= 111 us at 2.4 GHz
(f32r baseline: 343k = 143 us). ACT ~27 us/qb worst, DVE ~19 us/qb
worst, both under the PE's ~28 us/qb.
"""

import numpy as np
import ml_dtypes

import concourse.bass as bass  # noqa: F401  (AP types come through tile/bacc)
import concourse.tile as tile
from concourse import bacc, mybir
from concourse.bass_utils import run_bass_kernel_spmd

B, S, D = 4, 4096, 256
SQ = S // 2  # queries per core
NCORES = 8
F32 = mybir.dt.float32
BF16 = mybir.dt.bfloat16
F8 = mybir.dt.float8e4
U8 = mybir.dt.uint8
EXP_SCALE = 1.0 / 256.0  # 1/sqrt(D) folded with the 16x M scaling
E4M3 = ml_dtypes.float8_e4m3
DR = mybir.MatmulPerfMode.DoubleRow
ALU = mybir.AluOpType

# wp2b packed byte offsets (per partition)
WP2_WO, WP2_CVEC, WP2_BOB, WP2_IDENT, WP2_END = (0, 1024, 1152, 2176, 2432)


def _build(phases=3):
    nc = bacc.Bacc("TRN2", target_bir_lowering=False, debug=False,
                   num_devices=NCORES)

    # x^T hi/lo splits, chunk-stacked and packed: per partition row d_lo,
    # [xh chunk0 sk | xh chunk1 sk | xl chunk0 sk | xl chunk1 sk] fp8
    xpk_d = nc.dram_tensor("xpk", [128, 4 * S], F8, kind="ExternalInput").ap()
    # wpa: mh|ml|wvh|wvl packed fp8 [d_lo 128, (c 2, e 256)] each
    wpa_d = nc.dram_tensor("wpa", [128, 2048], U8, kind="ExternalInput").ap()
    # wp2b: wo|cvec|bob|ident (see WP2_* offsets)
    wp2b_d = nc.dram_tensor("wp2b", [128, WP2_END], U8,
                            kind="ExternalInput").ap()
    out = nc.dram_tensor("out", [SQ, D], F32, kind="ExternalOutput").ap()

    out_g = out.rearrange("(g j p) c -> g p j c", j=2, p=128)  # [8,128,2,256]
    xpk_g = xpk_d.rearrange("p (h c s) -> p h c s", h=2, c=2)

    with tile.TileContext(nc) as tc:
        with (
            tc.tile_pool(name="const", bufs=1) as cpool,
            tc.tile_pool(name="pt", bufs=10) as pt_pool,
            tc.tile_pool(name="pts", bufs=8) as pts_pool,
            tc.tile_pool(name="ovec", bufs=2) as ovec_pool,
            tc.tile_pool(name="fout", bufs=2) as fout_pool,
            tc.tile_pool(name="psmm", bufs=1, space="PSUM") as psmm,
            tc.tile_pool(name="psacc", bufs=1, space="PSUM") as psacc,
        ):
            # ---- input DMAs, ordered so A block 0 can start ~3 us in ----
            xpk = cpool.tile([128, 4 * S], F8, tag="xpk", name="xpk")
            xpk_4 = xpk[:].rearrange("p (h c s) -> p h c s", h=2, c=2)
            xht_3 = xpk[:, 0:2 * S].rearrange("p (c s) -> p c s", c=2)
            xlt_3 = xpk[:, 2 * S:4 * S].rearrange("p (c s) -> p c s", c=2)
            wpa = cpool.tile([128, 2048], U8, tag="wpa", name="wpa")
            wp2b = cpool.tile([128, WP2_END], U8, tag="wp2b", name="wp2b")

            nc.sync.dma_start(wpa[:], wpa_d)
            nc.sync.dma_start(xpk_4[:, :, :, 0:512], xpk_g[:, :, :, 0:512])
            nc.sync.dma_start(xpk_4[:, :, :, 512:SQ],
                              xpk_g[:, :, :, 512:SQ])
            nc.sync.dma_start(wp2b[:], wp2b_d)
            nc.sync.dma_start(xpk_4[:, :, :, SQ:S], xpk_g[:, :, :, SQ:S])

            mh = wpa[:, 0:512].bitcast(F8).rearrange("p (c e) -> p c e", c=2)
            ml = wpa[:, 512:1024].bitcast(F8).rearrange(
                "p (c e) -> p c e", c=2)
            wvh = wpa[:, 1024:1536].bitcast(F8).rearrange(
                "p (c e) -> p c e", c=2)
            wvl = wpa[:, 1536:2048].bitcast(F8).rearrange(
                "p (c e) -> p c e", c=2)
            wo_3 = wp2b[:, WP2_WO:WP2_CVEC].bitcast(BF16).rearrange(
                "p (c e) -> p c e", c=2)
            cvec = wp2b[:, WP2_CVEC:WP2_BOB].bitcast(F32)     # [128, 32]
            bob = wp2b[:, WP2_BOB:WP2_IDENT].bitcast(F32)     # [128, 256]
            ident = wp2b[:, WP2_IDENT:WP2_END].bitcast(BF16)  # [128, 128]

            ones16 = cpool.tile([128, 128], BF16, tag="ones16", name="ones16")
            # 16.0 in bf16 is 0x4180
            nc.vector.memset(ones16[:].bitcast(mybir.dt.uint16), 0x4180)

            # ---- persistent activations ----
            ahT = cpool.tile([128, 2 * SQ], F8, tag="ahT", name="ahT")
            alT = cpool.tile([128, 2 * SQ], F8, tag="alT", name="alT")
            ahT_3 = ahT[:].rearrange("p (c q) -> p c q", c=2)
            alT_3 = alT[:].rearrange("p (c q) -> p c q", c=2)
            v_sb = cpool.tile([128, 32 * D], BF16, tag="v", name="v")

            def emit_ablk_et(blk, et):
                """One e-tile of A^T q-block: 3-term fp8 DoubleRow matmuls
                plus hi/lo split eviction (ACT copy, DVE subtract)."""
                qsl = slice(blk * 512, (blk + 1) * 512)
                esl = slice(et * 128, (et + 1) * 128)
                pp = psmm.tile([128, 512], F32, tag="sc", name="sc", bufs=5)
                nc.tensor.matmul(pp[:], mh[:, :, esl], xht_3[:, :, qsl],
                                 start=True, stop=False, perf_mode=DR)
                nc.tensor.matmul(pp[:], ml[:, :, esl], xht_3[:, :, qsl],
                                 start=False, stop=False, perf_mode=DR)
                nc.tensor.matmul(pp[:], mh[:, :, esl], xlt_3[:, :, qsl],
                                 start=False, stop=True, perf_mode=DR)
                nc.scalar.copy(ahT_3[:, et, qsl], pp[:])
                nc.vector.tensor_sub(alT_3[:, et, qsl], pp[:],
                                     ahT_3[:, et, qsl])

            vparity = [0]

            def emit_vpair(stp, force_dve=False):
                """v16 for sk-tiles 2stp,2stp+1: 3-term fp8 DR, one bank."""
                vp = psmm.tile([128, 512], F32, tag="sc", name="sc", bufs=5)
                for half in range(2):
                    st = stp * 2 + half
                    ssl = slice(st * 128, (st + 1) * 128)
                    osl = slice(half * D, (half + 1) * D)
                    nc.tensor.matmul(vp[:, osl], xht_3[:, :, ssl], wvh[:],
                                     start=True, stop=False, perf_mode=DR)
                    nc.tensor.matmul(vp[:, osl], xlt_3[:, :, ssl], wvh[:],
                                     start=False, stop=False, perf_mode=DR)
                    nc.tensor.matmul(vp[:, osl], xht_3[:, :, ssl], wvl[:],
                                     start=False, stop=True, perf_mode=DR)
                dsl = v_sb[:, stp * 512:(stp + 1) * 512]
                if force_dve or vparity[0] % 2 == 0:
                    nc.vector.tensor_copy(dsl, vp[:])
                else:
                    nc.scalar.copy(dsl, vp[:])
                vparity[0] += 1

            # PE p-state warmup: the ramp to 2.4 GHz needs ~3 us of
            # continuous PE activity; burn the initial DMA wait on dummy
            # matmuls over the memset constant so real work starts at full
            # clock. Results land in the (still unused) accd bank.
            if phases >= 1:
                warm = psacc.tile([128, 512], F32, tag="accd", name="accd")
                for i in range(24):
                    nc.tensor.matmul(warm[:, 0:128], ones16[:],
                                     ones16[:], start=True, stop=True)
                emit_ablk_et(0, 0)
                emit_ablk_et(0, 1)
                emit_vpair(0)
                emit_vpair(1)
                emit_vpair(2)

            # ---- attention ----
            # The per-block tail (den^T/rec, numerator evictions, final
            # projection) is software-pipelined into the next block's first
            # iterations so its eviction latencies hide under scores/PV.
            def tail_part1(p, terminal=False):
                """Evictions of accd and the numerators; frees all psacc
                banks for the next block. den first: the PE transposes are
                its only consumer and come earliest."""
                den = ovec_pool.tile([128, 512], BF16, tag="den", name="den")
                nc.scalar.copy(den[:], p["accd"][:])
                p["den"] = den
                o0 = ovec_pool.tile([128, 512], BF16, tag="o0", name="o0")
                nc.scalar.copy(o0[:], p["acc"][0][:])
                o1 = ovec_pool.tile([128, 512], BF16, tag="o1", name="o1")
                nc.vector.tensor_copy(o1[:], p["acc"][1][:])
                p["o"] = (o0, o1)

            def tail_part2(p):
                """den^T on the PE (bf16), 1/den as a [128,4] strided read."""
                scd = psmm.tile([128, 512], F32, tag="sc", name="sc", bufs=5)
                scd_bf = scd[:, 0:256].bitcast(BF16)
                den = p["den"]
                for t4 in range(4):
                    nc.tensor.transpose(scd_bf[:, t4 * 128:(t4 + 1) * 128],
                                        den[:, t4 * 128:(t4 + 1) * 128],
                                        ident)
                rec = ovec_pool.tile([128, 4], F32, tag="rec", name="rec")
                nc.vector.reciprocal(
                    rec[:],
                    scd_bf.rearrange("p (b c) -> p b c", c=128)[:, :, 0])
                p["rec"] = rec

            def tail_part3(p):
                """fp = o Wo into the freed accd bank + one sc slot; evict
                with the fused 1/den scale and output bias; DMA out."""
                fp4a = psacc.tile([128, 512], F32, tag="accd", name="accd")
                scx = psmm.tile([128, 512], F32, tag="sc", name="sc", bufs=5)
                fp_slices = [fp4a[:, 0:256], fp4a[:, 256:512],
                             scx[:, 0:256], scx[:, 256:512]]
                o, rec = p["o"], p["rec"]
                for pair in range(2):
                    fo = fout_pool.tile([128, 2 * D], F32, tag="fout",
                                        name="fout")
                    for half in range(2):
                        t4 = pair * 2 + half
                        tsl = slice(t4 * 128, (t4 + 1) * 128)
                        fp = fp_slices[t4]
                        for e in range(2):
                            nc.tensor.matmul(fp, o[e][:, tsl], wo_3[:, e, :],
                                             start=(e == 0), stop=(e == 1))
                        nc.vector.scalar_tensor_tensor(
                            fo[:, half * D:(half + 1) * D], fp,
                            rec[:, t4:t4 + 1], bob,
                            op0=ALU.mult, op1=ALU.add)
                    nc.sync.dma_start(out_g[p["qb"] * 2 + pair],
                                      fo.rearrange("p (j c) -> p j c", j=2))

            # Producer/consumer skew: scores+exp run LEAD tiles ahead of the
            # PV+denominator consumer, across qb boundaries, so ACT's exp
            # pipeline never restarts from empty when a new block begins.
            LEAD = 2
            state = {"acc": None, "accd": None, "pending": None,
                     "ptq": [], "l1q": [], "l2q": [], "pc_defer": None}

            def consume(cqb, cst, pt):
                if cst == 0:
                    if state["pending"] is not None:
                        tail_part1(state["pending"])
                    state["acc"] = [
                        psacc.tile([128, 512], F32, tag=f"acc{e}",
                                   name=f"acc{e}") for e in range(2)]
                    state["accd"] = None
                acc = state["acc"]
                first, last = (cst == 0), (cst == 31)
                nc.tensor.matmul(acc[0][:], v_sb[:, cst * D:cst * D + 128],
                                 pt[:], start=first, stop=last)
                nc.tensor.matmul(acc[1][:],
                                 v_sb[:, cst * D + 128:(cst + 1) * D],
                                 pt[:], start=first, stop=last)
                p = state["pending"]
                if p is not None:
                    if cst == 1:
                        tail_part2(p)
                    elif cst == 2:
                        tail_part3(p)
                        state["pending"] = None
                # Denominator: incremental oct-tree sums on DVE (bf16 2x
                # mode) as pairs complete, then one 16.0-stationary matmul
                # per 8 tiles.
                ptq, l1q, l2q = state["ptq"], state["l1q"], state["l2q"]
                ptq.append(pt)
                if cst % 2 == 1:
                    ps = pts_pool.tile([128, 512], BF16, tag="pts",
                                       name="pts")
                    nc.vector.tensor_add(ps[:], ptq[-2][:], ptq[-1][:])
                    l1q.append(ps)
                if cst % 4 == 3:
                    ps = pts_pool.tile([128, 512], BF16, tag="pts",
                                       name="pts")
                    nc.vector.tensor_add(ps[:], l1q[-2][:], l1q[-1][:])
                    l2q.append(ps)
                if cst % 8 == 7:
                    ps = pts_pool.tile([128, 512], BF16, tag="pts",
                                       name="pts")
                    nc.vector.tensor_add(ps[:], l2q[-2][:], l2q[-1][:])
                    if state["accd"] is None:
                        state["accd"] = psacc.tile([128, 512], F32,
                                                   tag="accd", name="accd")
                    if cst == 31:
                        if state["pc_defer"] is not None:
                            nc.tensor.matmul(state["accd"][:], ones16[:],
                                             state["pc_defer"][:],
                                             start=False, stop=False)
                        nc.tensor.matmul(state["accd"][:], ones16[:], ps[:],
                                         start=False, stop=True)
                        state["pc_defer"] = None
                    else:
                        if state["pc_defer"] is not None:
                            nc.tensor.matmul(state["accd"][:], ones16[:],
                                             state["pc_defer"][:],
                                             start=(cst == 15), stop=False)
                        state["pc_defer"] = ps
                        if cst == 7:
                            state["accd_start"] = True
                    state["ptq"], state["l1q"], state["l2q"] = [], [], []
                if cst == 31:
                    state["pending"] = {"acc": acc, "accd": state["accd"],
                                        "qb": cqb}

            nqb = SQ // 512 if phases >= 2 else 0
            fifo = []
            for qb in range(nqb):
                qsl = slice(qb * 512, (qb + 1) * 512)
                for st in range(32):
                    # interleaved producer work for later consumers
                    if qb == 0 and st % 2 == 0 and st // 2 + 3 <= 15:
                        emit_vpair(st // 2 + 3)
                    if qb < 3 and st in (11, 21):
                        emit_ablk_et(qb + 1, 0 if st == 11 else 1)

                    ssl = slice(st * 128, (st + 1) * 128)
                    sp = psmm.tile([128, 512], F32, tag="sc", name="sc",
                                   bufs=5)
                    nc.tensor.matmul(sp[:], xht_3[:, :, ssl],
                                     ahT_3[:, :, qsl], start=True, stop=False,
                                     perf_mode=DR)
                    nc.tensor.matmul(sp[:], xht_3[:, :, ssl],
                                     alT_3[:, :, qsl], start=False,
                                     stop=False, perf_mode=DR)
                    nc.tensor.matmul(sp[:], xlt_3[:, :, ssl],
                                     ahT_3[:, :, qsl], start=False, stop=True,
                                     perf_mode=DR)
                    pt = pt_pool.tile([128, 512], BF16, tag="pt", name="pt",
                                      bufs=10)
                    nc.scalar.activation(pt[:], sp[:],
                                         mybir.ActivationFunctionType.Exp,
                                         scale=EXP_SCALE,
                                         bias=cvec[:, st:st + 1])
                    fifo.append((qb, st, pt))
                    if len(fifo) > LEAD:
                        consume(*fifo.pop(0))
            for item in fifo:
                consume(*item)

            if state["pending"] is not None and phases >= 2:
                # Terminal tail: nothing left to overlap with, so order by
                # dependency readiness: numerators evict first (fp matmuls
                # need only those), the denominator chain runs concurrently,
                # and each output row-tile DMAs as soon as it is scaled.
                p = state["pending"]
                o0 = ovec_pool.tile([128, 512], BF16, tag="o0", name="o0")
                nc.scalar.copy(o0[:], p["acc"][0][:])
                o1 = ovec_pool.tile([128, 512], BF16, tag="o1", name="o1")
                nc.vector.tensor_copy(o1[:], p["acc"][1][:])
                o = (o0, o1)
                den = ovec_pool.tile([128, 512], BF16, tag="den", name="den")
                nc.scalar.copy(den[:], p["accd"][:])
                fpa = psacc.tile([128, 512], F32, tag="acc0",
                                 name="acc0")
                fpb = psacc.tile([128, 512], F32, tag="acc1",
                                 name="acc1")
                fp_slices = [fpa[:, 0:256], fpa[:, 256:512],
                             fpb[:, 0:256], fpb[:, 256:512]]
                for t4 in range(4):
                    tsl = slice(t4 * 128, (t4 + 1) * 128)
                    for e in range(2):
                        nc.tensor.matmul(fp_slices[t4], o[e][:, tsl],
                                         wo_3[:, e, :],
                                         start=(e == 0), stop=(e == 1))
                scd = psmm.tile([128, 512], F32, tag="sc", name="sc", bufs=5)
                scd_bf = scd[:, 0:256].bitcast(BF16)
                for t4 in range(4):
                    nc.tensor.transpose(scd_bf[:, t4 * 128:(t4 + 1) * 128],
                                        den[:, t4 * 128:(t4 + 1) * 128],
                                        ident)
                rec = ovec_pool.tile([128, 4], F32, tag="rec", name="rec")
                nc.vector.reciprocal(
                    rec[:],
                    scd_bf.rearrange("p (b c) -> p b c", c=128)[:, :, 0])
                for pair in range(2):
                    fo = fout_pool.tile([128, 2 * D], F32, tag="fout",
                                        name="fout")
                    for half in range(2):
                        t4 = pair * 2 + half
                        nc.vector.scalar_tensor_tensor(
                            fo[:, half * D:(half + 1) * D], fp_slices[t4],
                            rec[:, t4:t4 + 1], bob,
                            op0=ALU.mult, op1=ALU.add)
                    nc.sync.dma_start(out_g[p["qb"] * 2 + pair],
                                      fo.rearrange("p (j c) -> p j c", j=2))

    nc.compile()
    return nc


_NC = None


def _get_nc():
    global _NC
    if _NC is None:
        _NC = _build()
    return _NC


class _Runner:
    """Cached jitted SPMD executor (run_bass_kernel_spmd rebuilds its jax
    closure every call, forcing a retrace; this traces once)."""

    def __init__(self, nc):
        import jax
        from jax.sharding import Mesh, PartitionSpec
        from jax.experimental.shard_map import shard_map
        from concourse import bass2jax, mybir as mb

        bass2jax.install_neuronx_cc_hook()
        self.jax = jax
        if not any("axon" in str(getattr(d, "platform", "")).lower()
                   or str(d).startswith("NC_")
                   for d in jax.devices()):
            # jax was initialized on another platform (e.g. cpu for the
            # reference); reset so the axon NeuronCores are visible.
            import jax._src.xla_bridge as xb
            jax.config.update("jax_platforms", None)
            xb._clear_backends()
            if hasattr(xb.get_backend, "cache_clear"):
                xb.get_backend.cache_clear()
            if not any("axon" in str(getattr(d, "platform", "")).lower()
                       or str(d).startswith("NC_")
                       for d in jax.devices()):
                jax.config.update("jax_platforms", "axon")
                xb._clear_backends()
                if hasattr(xb.get_backend, "cache_clear"):
                    xb.get_backend.cache_clear()
        partition_name = (nc.partition_id_tensor.name
                          if nc.partition_id_tensor else None)
        in_names, out_names, out_avals = [], [], []
        for alloc in nc.m.functions[0].allocations:
            if not isinstance(alloc, mb.MemoryLocationSet):
                continue
            name = alloc.memorylocations[0].name
            if alloc.kind == "ExternalInput":
                if name != partition_name:
                    in_names.append(name)
            elif alloc.kind == "ExternalOutput":
                out_names.append(name)
                out_avals.append(jax.core.ShapedArray(
                    tuple(alloc.tensor_shape), mb.dt.np(alloc.dtype)))
        self.in_names, self.out_names, self.out_avals = \
            in_names, out_names, out_avals
        n_params, n_outs = len(in_names), len(out_names)
        bind_in_names = in_names + out_names + (
            [partition_name] if partition_name else [])

        def _body(*args):
            operands = list(args)
            if partition_name is not None:
                operands.append(bass2jax.partition_id_tensor())
            outs = bass2jax._bass_exec_p.bind(
                *operands,
                out_avals=tuple(out_avals),
                in_names=tuple(bind_in_names),
                out_names=tuple(out_names),
                lowering_input_output_aliases=(),
                sim_require_finite=True,
                sim_require_nnan=True,
                nc=nc,
            )
            return tuple(outs)

        devices = jax.devices()[:NCORES]
        mesh = Mesh(np.asarray(devices), ("core",))
        spec = (PartitionSpec("core"),) * (n_params + n_outs)
        self.fn = jax.jit(
            shard_map(_body, mesh=mesh, in_specs=spec,
                      out_specs=(PartitionSpec("core"),) * n_outs,
                      check_rep=False),
            donate_argnums=tuple(range(n_params, n_params + n_outs)),
            keep_unused=True,
        )

    def run(self, in_maps):
        concat_in = [
            np.concatenate([np.asarray(m[n]) for m in in_maps], axis=0)
            for n in self.in_names
        ]
        concat_zeros = [
            np.zeros((NCORES * a.shape[0], *a.shape[1:]), a.dtype)
            for a in self.out_avals
        ]
        outs = self.fn(*concat_in, *concat_zeros)
        return [
            {n: np.asarray(outs[i]).reshape(NCORES, *self.out_avals[i].shape)[c]
             for i, n in enumerate(self.out_names)}
            for c in range(NCORES)
        ]


_RUNNER = None


def _get_runner():
    global _RUNNER
    if _RUNNER is None:
        _RUNNER = _Runner(_get_nc())
    return _RUNNER


def _split8(a):
    """fp8 e4m3 hi/lo split: a ~= hi + lo elementwise."""
    hi = np.asarray(a, dtype=E4M3)
    lo = np.asarray(a.astype(np.float32) - hi.astype(np.float32), dtype=E4M3)
    return hi, lo


def _dstack(a):
    """[256, N] (d-major) -> [128, 2*N] chunk-stacked: row d_lo holds
    (chunk 0 cols, chunk 1 cols)."""
    n = a.shape[1]
    return np.ascontiguousarray(
        a.reshape(2, 128, n).transpose(1, 0, 2).reshape(128, 2 * n))


def make_in_maps(inputs):
    x = np.asarray(inputs["x"], dtype=np.float32)
    Wq = np.asarray(inputs["Wq"], dtype=np.float32)
    Wk = np.asarray(inputs["Wk"], dtype=np.float32)
    Wv = np.asarray(inputs["Wv"], dtype=np.float32)
    Wo = np.asarray(inputs["Wo"], dtype=np.float32)
    bq = np.asarray(inputs["bq"], dtype=np.float32)
    bv = np.asarray(inputs["bv"], dtype=np.float32)
    bo = np.asarray(inputs["bo"], dtype=np.float32)
    # bk drops out of softmax (per-query constant). bq only survives through
    # the per-key term x_k . (Wk bq), applied as an exp bias. bv folds into
    # the output bias (attention rows sum to 1).
    u8 = np.uint8
    M16 = (16.0 * (Wq @ Wk.T)).astype(np.float32)
    mh, ml = _split8(M16)
    wvh, wvl = _split8((16.0 * Wv).astype(np.float32))
    wpa = np.ascontiguousarray(np.concatenate(
        [_dstack(mh).view(u8), _dstack(ml).view(u8),
         _dstack(wvh).view(u8), _dstack(wvl).view(u8)], axis=1))
    wo_b = _dstack(np.asarray(Wo, dtype=ml_dtypes.bfloat16))
    bob = np.tile((bv @ Wo + bo).astype(np.float32)[None, :], (128, 1))
    ident = np.eye(128, dtype=ml_dtypes.bfloat16)
    wkbq = (Wk @ bq).astype(np.float32)
    in_maps = []
    for c in range(NCORES):
        b, h = divmod(c, 2)
        # Rotate the batch so this core's queries are rows 0..SQ-1; keys and
        # values see all rows either way (softmax is key-order invariant).
        xb = x[b] if h == 0 else np.ascontiguousarray(
            np.concatenate([x[b, SQ:], x[b, :SQ]]))
        xh, xl = _split8(xb)
        cvec = np.ascontiguousarray(
            (xb @ wkbq).astype(np.float32).reshape(32, 128).T)
        wp2b = np.ascontiguousarray(np.concatenate(
            [wo_b.view(u8), cvec.view(u8), bob.view(u8), ident.view(u8)],
            axis=1))
        assert wp2b.shape == (128, WP2_END)
        in_maps.append({
            "xpk": np.concatenate(
                [_dstack(np.ascontiguousarray(xh.T)),
                 _dstack(np.ascontiguousarray(xl.T))], axis=1),
            "wpa": wpa,
            "wp2b": wp2b,
        })
    return in_maps


def kernel(**inputs):
    try:
        runner = _get_runner()
    except Exception:
        runner = None
    in_maps = make_in_maps(inputs)
    results = None
    if runner is not None:
        try:
            results = runner.run(in_maps)
        except Exception:
            results = None
    if results is None:
        results = run_bass_kernel_spmd(
            _get_nc(), in_maps, core_ids=list(range(NCORES))).results
    outp = np.empty((B, S, D), dtype=np.float32)
    for c in range(NCORES):
        b, h = divmod(c, 2)
        outp[b, h * SQ:(h + 1) * SQ] = results[c]["out"]
    return outp
